# revision 1
# baseline (speedup 1.0000x reference)
"""CenterLoss kernel for Trainium2 (Bass/Tile), 8 NeuronCores, fp16 inputs.

Primary strategy (class-sharded collapsed form):
  Shard the 751 classes across 8 cores (94/core); the host routes each batch
  row to the core owning its label (a permuted batch shard), pads each shard
  to a common tile count and localizes labels. The reference's
  clip(dist, 1e-12, 1e12) is provably inactive for this distribution
  (dist in [3542, 4722]), so the mean collapses to
      sum_b ||x_b||^2 + sum_c n_c ||C_c||^2 - 2 sum_c <S_c, C_c>
  with S = onehot^T X the per-class segment sum. Per 128-row tile the device
  does: a batched x DMA, a one-hot build (DVE is_equal), a segment-sum
  matmul on PE accumulating in PSUM, and a square+accumulate sweep
  alternating ScalarE/DVE. fp16 inputs (host casts; final error ~1e-6),
  f32 accumulation everywhere. ~44-48 us on 8 cores.

Fallback strategy (indirect gather): batch-shard rows; per tile gather the
128 label centers from DRAM via indirect DMA, DVE subtract, ScalarE
square+accumulate, on-device clip+reduce. ~72 us, very stable.

HW bring-up notes: tensor_tensor_reduce crashes the device
(NRT_EXEC_UNIT_UNRECOVERABLE); scalar_tensor_tensor computes the same fused
multiply+sum and is stable. The runtime also crashes sporadically on some
kernels, hence the retry/fallback ladder.
"""

import os
import sys

import numpy as np

sys.path.insert(0, "/opt/trn_rl_repo")

import concourse.bass as bass
import concourse.bass_isa as bass_isa
import concourse.tile as tile
from concourse import bacc, mybir
from concourse.bass_utils import run_bass_kernel_spmd

N_CORES = 8
B = 16384
F = 2048
C = 751
P = 128
CPC = 94  # classes per core (8 * 94 = 752 >= 751)

LAST_RESULTS = None
_cached = {}


def _install_ntff_shim():
    """Make trace=True work in containers whose antenv lacks axon_hooks."""
    import types

    try:
        import antenv.axon_hooks  # noqa: F401
        return
    except ImportError:
        pass
    try:
        from trn_agent_boot.trn_boot import _ntff_profile_via_ctypes

        hook = _ntff_profile_via_ctypes("/opt/axon/libaxon_pjrt.so")
        mod = types.ModuleType("antenv.axon_hooks")
        mod.get_axon_ntff_profile_hook = lambda: hook
        sys.modules["antenv.axon_hooks"] = mod
        import concourse.bass_utils as _bu

        _bu.upload_artifacts = lambda tmpdir: tmpdir
    except Exception:
        pass


def _build_h(n_tiles, group=2, sweep="ADADADADADADADADA"):
    """Class-sharded collapsed-form kernel (primary)."""
    cap = n_tiles * P
    n_groups = -(-n_tiles // group)
    nc = bacc.Bacc("TRN2", target_bir_lowering=False, debug=False)

    f32 = mybir.dt.float32
    f16 = mybir.dt.float16
    x_d = nc.dram_tensor("x", [cap, F], f16, kind="ExternalInput").ap()
    labs_d = nc.dram_tensor("labs", [P, n_tiles], f32,
                            kind="ExternalInput").ap()
    cnt_d = nc.dram_tensor("counts", [P, 1], f32, kind="ExternalInput").ap()
    cs_d = nc.dram_tensor("cslice", [P, F], f16, kind="ExternalInput").ap()
    iota_d = nc.dram_tensor("iota", [P, P], f16, kind="ExternalInput").ap()
    out_d = nc.dram_tensor("out", [1, 1], f32, kind="ExternalOutput").ap()

    xr = x_d.rearrange("(n p) f -> p n f", p=P)
    NACC = n_groups + 5  # xx per group | counts*csq | 4 chunks of -2*<S,C>

    with tile.TileContext(nc) as tc:
        with (
            tc.tile_pool(name="xp", bufs=1) as xp,
            tc.tile_pool(name="oh", bufs=4) as ohp,
            tc.tile_pool(name="sc32", bufs=2) as sc32,
            tc.tile_pool(name="sc16", bufs=2) as sc16,
            tc.tile_pool(name="small", bufs=1) as sp,
            tc.tile_pool(name="psum", bufs=1, space="PSUM") as pp,
        ):
            acc = sp.tile([P, NACC], f32)
            S = [pp.tile([P, 512], f32, tag=f"S{j}", name=f"S{j}")
                 for j in range(4)]

            # tiny constants first (the HWDGE queue is in-order; labs/iota
            # feed the onehots and must precede the big x transfers)
            labs = sp.tile([P, n_tiles], f32)
            nc.sync.dma_start(out=labs[:], in_=labs_d[:, :])
            iota = sp.tile([P, P], f16)
            nc.sync.dma_start(out=iota[:], in_=iota_d[:, :])

            xbufs, xgroups = [], []
            for g in range(n_groups):
                g0 = g * group
                gn = min(group, n_tiles - g0)
                xg = xp.tile([P, gn, F], f16, name=f"xg{g}", tag=f"xg{g}")
                nc.sync.dma_start(out=xg[:], in_=xr[:, g0:g0 + gn, :])
                xgroups.append((xg, gn))
                for s in range(gn):
                    xbufs.append(xg[:, s, :])

            # cs/cnt only feed the tail
            cs = sp.tile([P, F], f16)
            nc.sync.dma_start(out=cs[:], in_=cs_d[:, :])
            cnt = sp.tile([P, 1], f32)
            nc.sync.dma_start(out=cnt[:], in_=cnt_d[:, :])

            for t in range(n_tiles):
                xt = xbufs[t]
                ohm = ohp.tile([P, P], f16)
                nc.vector.tensor_scalar(
                    out=ohm[:], in0=iota[:], scalar1=labs[:, t:t + 1],
                    scalar2=None, op0=mybir.AluOpType.is_equal)
                for j in range(4):
                    nc.tensor.matmul(
                        S[j][:], lhsT=ohm[:],
                        rhs=xt[:, 512 * j:512 * (j + 1)],
                        start=(t == 0), stop=(t == n_tiles - 1))

            # square+accumulate once per DMA group, alternating engines
            for g, (xg, gn) in enumerate(xgroups):
                flat = xg[:].rearrange("p n f -> p (n f)")
                if sweep[g % len(sweep)] == "A":
                    sq = sc32.tile([P, gn * F], f32, tag="sq", name="sq")
                    nc.scalar.activation(
                        out=sq[:], in_=flat,
                        func=mybir.ActivationFunctionType.Square,
                        accum_out=acc[:, g:g + 1])
                else:
                    sq16 = sc16.tile([P, gn * F], f16, tag="sq16",
                                     name="sq16")
                    nc.vector.scalar_tensor_tensor(
                        out=sq16[:], in0=flat, scalar=1.0, in1=flat,
                        op0=mybir.AluOpType.mult, op1=mybir.AluOpType.mult,
                        accum_out=acc[:, g:g + 1])

            csq = sp.tile([P, 1], f32)
            csqs = sc32.tile([P, F], f32)
            nc.scalar.activation(
                out=csqs[:], in_=cs[:],
                func=mybir.ActivationFunctionType.Square, accum_out=csq[:])
            nc.vector.tensor_tensor(
                out=acc[:, n_groups:n_groups + 1], in0=csq[:], in1=cnt[:],
                op=mybir.AluOpType.mult)

            for j in range(4):
                scj = sc32.tile([P, 512], f32, name="scj")
                nc.vector.scalar_tensor_tensor(
                    out=scj[:], in0=S[j][:], scalar=-2.0,
                    in1=cs[:, 512 * j:512 * (j + 1)],
                    op0=mybir.AluOpType.mult, op1=mybir.AluOpType.mult,
                    accum_out=acc[:, n_groups + 1 + j:n_groups + 2 + j])

            colsum = sp.tile([P, 1], f32)
            nc.vector.tensor_reduce(
                out=colsum[:], in_=acc[:], axis=mybir.AxisListType.X,
                op=mybir.AluOpType.add)
            tot = sp.tile([P, 1], f32)
            nc.gpsimd.partition_all_reduce(
                tot[:], colsum[:], channels=P, reduce_op=bass_isa.ReduceOp.add)
            nc.sync.dma_start(out=out_d[:, :], in_=tot[0:1, 0:1])

    nc.compile()
    return nc


def _inputs_h(x16, c16, labels):
    idxs, lims = [], []
    for k in range(N_CORES):
        lo, hi = k * CPC, min((k + 1) * CPC, C)
        idx = np.nonzero((labels >= lo) & (labels < hi))[0]
        idxs.append(idx)
        lims.append((lo, hi))
    n_tiles = max(1, -(-max(len(i) for i in idxs) // P))
    cap = n_tiles * P
    iota_full = np.broadcast_to(
        np.arange(P, dtype=np.float16)[None, :], (P, P)).copy()

    in_maps = []
    for k in range(N_CORES):
        lo, hi = lims[k]
        idx = idxs[k]
        n_k = len(idx)
        xc = np.zeros((cap, F), np.float16)
        xc[:n_k] = x16[idx]
        ll = np.zeros(cap, np.float32)
        ll[:n_k] = (labels[idx] - lo).astype(np.float32)
        cnt = np.zeros((P, 1), np.float32)
        cnt[:hi - lo, 0] = np.bincount(labels[idx] - lo, minlength=hi - lo)
        cslice = np.zeros((P, F), np.float16)
        cslice[:hi - lo] = c16[lo:hi]
        in_maps.append({
            "x": xc,
            "labs": np.ascontiguousarray(ll.reshape(n_tiles, P).T),
            "counts": cnt,
            "cslice": cslice,
            "iota": iota_full,
        })
    return n_tiles, in_maps


def _build_a():
    """Batch-sharded indirect-gather kernel (fallback)."""
    b_local = B // N_CORES
    n_tiles = b_local // P
    nc = bacc.Bacc("TRN2", target_bir_lowering=False, debug=False)

    f32 = mybir.dt.float32
    f16 = mybir.dt.float16
    x_d = nc.dram_tensor("x", [b_local, F], f16, kind="ExternalInput").ap()
    lab_d = nc.dram_tensor("labels", [P, n_tiles], mybir.dt.int32,
                           kind="ExternalInput").ap()
    cen_d = nc.dram_tensor("centers", [C, F], f16, kind="ExternalInput").ap()
    out_d = nc.dram_tensor("out", [1, 1], f32, kind="ExternalOutput").ap()

    with tile.TileContext(nc) as tc:
        with (
            tc.tile_pool(name="xp", bufs=3) as xp,
            tc.tile_pool(name="gp", bufs=3) as gp,
            tc.tile_pool(name="dp", bufs=2) as dp,
            tc.tile_pool(name="sq", bufs=2) as sqp,
            tc.tile_pool(name="small", bufs=1) as sp,
        ):
            labs = sp.tile([P, n_tiles], mybir.dt.int32)
            nc.sync.dma_start(out=labs[:], in_=lab_d[:, :])
            acc = sp.tile([P, n_tiles], f32)

            for i in range(n_tiles):
                xt = xp.tile([P, F], f16)
                nc.sync.dma_start(out=xt[:], in_=x_d[i * P:(i + 1) * P, :])
                gt = gp.tile([P, F], f16)
                nc.gpsimd.indirect_dma_start(
                    out=gt[:], out_offset=None, in_=cen_d[:],
                    in_offset=bass.IndirectOffsetOnAxis(
                        ap=labs[:, i:i + 1], axis=0))
                diff = dp.tile([P, F], f16)
                nc.vector.tensor_tensor(
                    out=diff[:], in0=xt[:], in1=gt[:],
                    op=mybir.AluOpType.subtract)
                sqt = sqp.tile([P, F], f32)
                nc.scalar.activation(
                    out=sqt[:], in_=diff[:],
                    func=mybir.ActivationFunctionType.Square,
                    accum_out=acc[:, i:i + 1])

            nc.vector.tensor_scalar_max(acc[:], acc[:], 1e-12)
            nc.vector.tensor_scalar_min(acc[:], acc[:], 1e12)
            colsum = sp.tile([P, 1], f32)
            nc.vector.tensor_reduce(
                out=colsum[:], in_=acc[:], axis=mybir.AxisListType.X,
                op=mybir.AluOpType.add)
            total = sp.tile([P, 1], f32)
            nc.gpsimd.partition_all_reduce(
                total[:], colsum[:], channels=P,
                reduce_op=bass_isa.ReduceOp.add)
            nc.sync.dma_start(out=out_d[:, :], in_=total[0:1, 0:1])

    nc.compile()
    return nc


def _run_h(x16, c16, labels):
    global LAST_RESULTS
    n_tiles, in_maps = _inputs_h(x16, c16, labels)
    key = ("h", n_tiles)
    if key not in _cached:
        _cached[key] = _build_h(n_tiles)
    res = run_bass_kernel_spmd(_cached[key], in_maps,
                               core_ids=list(range(N_CORES)))
    LAST_RESULTS = res
    total = sum(float(res.results[k]["out"][0, 0]) for k in range(N_CORES))
    return total / B


def _run_a(x16, c16, labels):
    global LAST_RESULTS
    b_local = B // N_CORES
    n_tiles = b_local // P
    if "a" not in _cached:
        _cached["a"] = _build_a()
    lab32 = labels.astype(np.int32).reshape(N_CORES, n_tiles, P)
    in_maps = []
    for c in range(N_CORES):
        in_maps.append({
            "x": np.ascontiguousarray(x16[c * b_local:(c + 1) * b_local]),
            "labels": np.ascontiguousarray(lab32[c].T),
            "centers": c16,
        })
    res = run_bass_kernel_spmd(_cached["a"], in_maps,
                               core_ids=list(range(N_CORES)))
    LAST_RESULTS = res
    total = sum(float(res.results[k]["out"][0, 0]) for k in range(N_CORES))
    return total / B


def kernel(x, labels, centers):
    x16 = np.asarray(x, dtype=np.float32).astype(np.float16)
    c16 = np.asarray(centers, dtype=np.float32).astype(np.float16)
    labels = np.asarray(labels).astype(np.int64)

    if os.environ.get("BASS_TRACE"):
        _install_ntff_shim()

    # primary (2 attempts) -> stable fallback kernel (2 attempts) -> host.
    # The runtime sporadically reports NRT_EXEC_UNIT_UNRECOVERABLE; a rerun
    # usually succeeds.
    attempts = [_run_h, _run_h, _run_a, _run_a]
    last_err = None
    for fn in attempts:
        try:
            total = fn(x16, c16, labels)
            return np.asarray(total, dtype=np.float32)
        except Exception as e:  # noqa: BLE001
            last_err = e
            sys.stderr.write(f"kernel attempt failed ({type(e).__name__}); "
                             f"retrying\n")

    # last resort: host compute (correct, but no device timing)
    sys.stderr.write(f"all device attempts failed: {last_err}\n")
    g = c16[labels].astype(np.float32)
    diff = x16.astype(np.float32) - g
    dist = np.clip((diff * diff).sum(1), 1e-12, 1e12)
    return np.asarray(dist.mean(), dtype=np.float32)

